# revision 5
# baseline (speedup 1.0000x reference)
"""Trainium2 Bass kernel for the LSTM+dense+softmax model.

Model (see reference): x[T=512, B=256, IN=256] -> LSTM(H=128) last hidden
-> dense(OUT=1000) -> softmax. Data-parallel over batch across 8 cores
(32 batch elements per core), weights replicated.

Layout: recurrent state is kept transposed [H=128 partitions, batch] so the
per-step W_hh matmuls, gate nonlinearities and cell update all run at full
partition width with no transposes. Gate pre-activations for 4 consecutive
steps share one PSUM bank: W_ih*x contributions (+bias) are accumulated
ahead of time, W_hh*h is added when the step arrives, and ScalarE applies
sigmoid/tanh directly out of PSUM.
"""

import numpy as np

import concourse.bacc as bacc
import concourse.mybir as mybir
import concourse.tile as tile
from concourse.bass_utils import run_bass_kernel_spmd

SEQ = 512
B = 256
IN = 256
H = 128
OUT = 1000
N_CORES = 8
BC = B // N_CORES  # 32 batch per core
KT = IN // H  # 2 k-tiles for the input projection
G4 = 4  # gate order in this kernel: i, f, o, g  (torch order i,f,g,o)
PERM = [0, 1, 3, 2]  # torch gate block -> our gate slot
SPB = 4  # steps per PSUM bank group (4*4*32 fp32 = one 2KB bank)
AHEAD = 4  # bank groups of x-projection lookahead
CH = 32  # timesteps per streamed x chunk

F32 = mybir.dt.float32

_CACHE = {}


def _build(T):
    ngrp = T // SPB
    ch = min(CH, T)
    nc = bacc.Bacc("TRN2", target_bir_lowering=False, debug=False)

    xT = nc.declare_dram_parameter("xT", [H, KT, T, BC], F32, isOutput=False)
    whhT = nc.declare_dram_parameter("whhT", [H, G4, H], F32, isOutput=False)
    wihT = nc.declare_dram_parameter("wihT", [H, KT, G4, H], F32, isOutput=False)
    bias4 = nc.declare_dram_parameter("bias4", [G4, H], F32, isOutput=False)
    ind4 = nc.declare_dram_parameter("ind4", [G4, SPB * G4 * BC], F32, isOutput=False)
    wdT = nc.declare_dram_parameter("wdT", [H, OUT], F32, isOutput=False)
    bd = nc.declare_dram_parameter("bd", [1, OUT], F32, isOutput=False)
    out = nc.declare_dram_parameter("out", [BC, OUT], F32, isOutput=True)

    NSPLIT = 512  # dense tail: first PSUM bank columns
    NREST = OUT - NSPLIT

    with tile.TileContext(nc) as tc:
        with (
            tc.tile_pool(name="const", bufs=1) as constp,
            tc.tile_pool(name="xs", bufs=3) as xpool,
            tc.tile_pool(name="state", bufs=1) as state,
            tc.tile_pool(name="work", bufs=3) as work,
        ):
            whh_s = constp.tile([H, G4, H], F32)
            wih_s = constp.tile([H, KT, G4, H], F32)
            bias_s = constp.tile([G4, H], F32)
            ind_s = constp.tile([G4, SPB * G4 * BC], F32)
            wd_s = constp.tile([H, OUT], F32)
            bd_s = constp.tile([1, OUT], F32)
            ones1 = constp.tile([1, BC], F32)
            nc.gpsimd.dma_start(whh_s[:], whhT[:])
            nc.gpsimd.dma_start(wih_s[:], wihT[:])
            nc.gpsimd.dma_start(bias_s[:], bias4[:])
            nc.gpsimd.dma_start(ind_s[:], ind4[:])
            nc.gpsimd.dma_start(wd_s[:], wdT[:])
            nc.gpsimd.dma_start(bd_s[:], bd[:])
            nc.vector.memset(ones1[:], 1.0)

            # persistent state: h transposed [H, BC]; gc = [tanh(g) | c]
            hT = state.tile([H, BC], F32)
            gc = state.tile([H, 2 * BC], F32)
            nc.vector.memset(hT[:], 0.0)
            nc.vector.memset(gc[:], 0.0)

            nchunk = (T + ch - 1) // ch
            xtiles = [None] * nchunk

            def ensure_chunk(ci):
                if xtiles[ci] is None:
                    xt = xpool.tile([H, KT, ch, BC], F32)
                    nc.gpsimd.dma_start(
                        xt[:], xT[:, :, ci * ch : (ci + 1) * ch, :]
                    )
                    xtiles[ci] = xt

            with tc.tile_pool(name="psum", bufs=AHEAD + 2, space="PSUM") as psump:
                pstiles = [None] * ngrp

                def emit_xproj(g):
                    # accumulate W_ih*x (+ bias) for the 4 steps of group g
                    t0 = g * SPB
                    ci = t0 // ch
                    ensure_chunk(ci)
                    xt = xtiles[ci]
                    s0 = t0 - ci * ch
                    ps = psump.tile([H, SPB, G4, BC], F32)
                    pstiles[g] = ps
                    # bias first: the ONE start=True matmul covering the whole
                    # bank (start=True clears has_written bank-wide, so it must
                    # be the single first writer; everything after accumulates)
                    nc.tensor.matmul(
                        ps[:].rearrange("p a g b -> p (a g b)"),
                        bias_s[:],
                        ind_s[:],
                        start=True,
                        stop=False,
                        skip_group_check=True,
                    )
                    for gi in range(G4):
                        for kt in range(KT):
                            nc.tensor.matmul(
                                ps[:, :, gi, :],
                                wih_s[:, kt, gi, :],
                                xt[:, kt, s0 : s0 + SPB, :],
                                start=False,
                                stop=False,
                                skip_group_check=True,
                            )

                for g in range(min(AHEAD, ngrp)):
                    emit_xproj(g)

                for g in range(ngrp):
                    if g + AHEAD < ngrp:
                        emit_xproj(g + AHEAD)
                    ps = pstiles[g]
                    for s in range(SPB):
                        # W_hh * h into the gate bank (critical path)
                        for gi in range(G4):
                            nc.tensor.matmul(
                                ps[:, s, gi, :],
                                whh_s[:, gi, :],
                                hT[:],
                                start=False,
                                stop=(gi == G4 - 1),
                                skip_group_check=True,
                            )
                        sig3 = work.tile([H, 3 * BC], F32)
                        prod = work.tile([H, 2 * BC], F32)
                        tct = work.tile([H, BC], F32)
                        nc.scalar.activation(
                            sig3[:].rearrange("p (g b) -> p g b", g=3),
                            ps[:, s, 0:3, :],
                            mybir.ActivationFunctionType.Sigmoid,
                        )
                        nc.scalar.activation(
                            gc[:, 0:BC],
                            ps[:, s, 3, :],
                            mybir.ActivationFunctionType.Tanh,
                        )
                        # prod = [i*g | f*c]
                        nc.vector.tensor_mul(
                            prod[:], sig3[:, 0 : 2 * BC], gc[:]
                        )
                        # c = i*g + f*c
                        nc.vector.tensor_add(
                            gc[:, BC : 2 * BC], prod[:, 0:BC], prod[:, BC : 2 * BC]
                        )
                        nc.scalar.activation(
                            tct[:],
                            gc[:, BC : 2 * BC],
                            mybir.ActivationFunctionType.Tanh,
                        )
                        nc.vector.tensor_mul(hT[:], sig3[:, 2 * BC : 3 * BC], tct[:])
                    pstiles[g] = None

            # dense + softmax tail
            with tc.tile_pool(name="psd", bufs=2, space="PSUM") as psumd:
                lA = psumd.tile([BC, NSPLIT], F32)
                lB = psumd.tile([BC, NREST], F32)
                nc.tensor.matmul(
                    lA[:], hT[:], wd_s[:, 0:NSPLIT], start=True, stop=False,
                    skip_group_check=True,
                )
                nc.tensor.matmul(
                    lA[:], ones1[:], bd_s[:, 0:NSPLIT], start=False, stop=True,
                    skip_group_check=True,
                )
                nc.tensor.matmul(
                    lB[:], hT[:], wd_s[:, NSPLIT:OUT], start=True, stop=False,
                    skip_group_check=True,
                )
                nc.tensor.matmul(
                    lB[:], ones1[:], bd_s[:, NSPLIT:OUT], start=False, stop=True,
                    skip_group_check=True,
                )
                mA = work.tile([BC, 1], F32)
                mB = work.tile([BC, 1], F32)
                mneg = work.tile([BC, 1], F32)
                sA = work.tile([BC, 1], F32)
                sB = work.tile([BC, 1], F32)
                stot = work.tile([BC, 1], F32)
                rec = work.tile([BC, 1], F32)
                sm = work.tile([BC, OUT], F32)
                nc.vector.reduce_max(mA[:], lA[:], axis=mybir.AxisListType.X)
                nc.vector.reduce_max(mB[:], lB[:], axis=mybir.AxisListType.X)
                nc.vector.tensor_max(mA[:], mA[:], mB[:])
                nc.vector.tensor_scalar_mul(mneg[:], mA[:], -1.0)
                nc.scalar.activation(
                    sm[:, 0:NSPLIT], lA[:], mybir.ActivationFunctionType.Exp,
                    bias=mneg[:], accum_out=sA[:],
                )
                nc.scalar.activation(
                    sm[:, NSPLIT:OUT], lB[:], mybir.ActivationFunctionType.Exp,
                    bias=mneg[:], accum_out=sB[:],
                )
                nc.vector.tensor_add(stot[:], sA[:], sB[:])
                nc.vector.reciprocal(rec[:], stot[:])
                nc.vector.tensor_scalar_mul(sm[:], sm[:], rec[:])
                nc.gpsimd.dma_start(out[:], sm[:])

    nc.compile()
    return nc


def _get_nc(T):
    if T not in _CACHE:
        _CACHE[T] = _build(T)
    return _CACHE[T]


def prep_inputs(x, w_ih, w_hh, b_ih, b_hh, w_dense, b_dense):
    T = x.shape[0]
    x = np.ascontiguousarray(x, dtype=np.float32)
    # xT[k, kt, t, b] = x[t, b, kt*128+k]
    xt_all = np.ascontiguousarray(
        x.reshape(T, B, KT, H).transpose(3, 2, 0, 1)
    )
    whhT = np.ascontiguousarray(
        w_hh.reshape(4, H, H)[PERM].transpose(2, 0, 1), dtype=np.float32
    )
    wihT = np.ascontiguousarray(
        w_ih.reshape(4, H, KT, H)[PERM].transpose(3, 2, 0, 1), dtype=np.float32
    )
    bias4 = np.ascontiguousarray(
        (b_ih + b_hh).reshape(4, H)[PERM], dtype=np.float32
    )
    # ind4[g, n] for n = s*(G4*BC) + gq*BC + b  -> 1.0 iff gq == g
    ind4 = np.zeros((G4, SPB * G4 * BC), dtype=np.float32)
    nidx = np.arange(SPB * G4 * BC)
    gq = (nidx // BC) % G4
    for g in range(G4):
        ind4[g, gq == g] = 1.0
    wdT = np.ascontiguousarray(w_dense.T, dtype=np.float32)
    bd = np.ascontiguousarray(b_dense.reshape(1, OUT), dtype=np.float32)

    in_maps = []
    for c in range(N_CORES):
        in_maps.append(
            {
                "xT": np.ascontiguousarray(xt_all[:, :, :, c * BC : (c + 1) * BC]),
                "whhT": whhT,
                "wihT": wihT,
                "bias4": bias4,
                "ind4": ind4,
                "wdT": wdT,
                "bd": bd,
            }
        )
    return in_maps


def kernel(x, w_ih, w_hh, b_ih, b_hh, w_dense, b_dense):
    x = np.asarray(x)
    T = x.shape[0]
    nc = _get_nc(T)
    in_maps = prep_inputs(
        np.asarray(x), np.asarray(w_ih), np.asarray(w_hh),
        np.asarray(b_ih), np.asarray(b_hh),
        np.asarray(w_dense), np.asarray(b_dense),
    )
    res = run_bass_kernel_spmd(nc, in_maps, list(range(N_CORES)))
    return np.concatenate(
        [res.results[c]["out"] for c in range(N_CORES)], axis=0
    ).astype(np.float32)
